# revision 55
# baseline (speedup 1.0000x reference)
"""CalderaLinear fused kernel for 8 Trainium2 NeuronCores.

Math (reference): y = x @ Q^T + (x @ R^T) @ L^T + bias, with Q/L/R groupwise
int-dequantized (codes 0..15, group size 128).

Strategy:
  * Column-parallel over d_out: core c owns out-features [c*512, (c+1)*512).
  * On each core, first build W_c = Q_c^T + R^T @ L_c^T  ([d_in, 512]) on-chip:
    R and L^T are dequantized with DVE multiplies (codes and pre-broadcast
    scales arrive as exact bf16), R^T L^T accumulates on the PE into PSUM, and
    dequantized Q^T is added during PSUM eviction into the resident W tile.
  * Then y_c = x @ W_c + bias_c: x streams through as 128x512 pre-tiled
    blocks (host-side retiling gives one contiguous DMA per tile), W_c stays
    SBUF-resident, PSUM accumulates over the 32 k-tiles, bias is fused into
    the PSUM eviction.
  * All W-build operands are packed host-side into one contiguous
    [128, 51200] blob so the build phase streams in as a handful of large
    DMAs (the per-tensor version paid ~2.5us of cold-queue latency per small
    DMA at kernel start).
  * Host side only reshapes/transposes/casts and concatenates the 8 output
    shards: all dequant + matmul math runs on the NeuronCores.

Compute dtype is bf16 (codes 0..15 are exact; rel-err ~3e-3 vs fp32
reference, dominated by bf16 rounding of x and W). Set CALDERA_DTYPE=float32r
for the reduced-precision-fp32 PE mode (~2e-4 rel-err, ~15% slower).
"""

import os
import numpy as np
import ml_dtypes

P = 128
D_IN = 4096
D_OUT = 4096
TOK = 8192
RANK = 256
NCORES = 8
OC = D_OUT // NCORES      # 512 out features per core
KT = D_IN // P            # 32 contraction tiles
MS = 512                  # token slab
NS = TOK // MS            # 16 slabs
SUB = MS // P             # 4 psum sub-tiles per slab
KG = D_IN // 128          # 32 scale groups along d_in
RG = RANK // 128          # 2 scale groups along rank

# ---- W-build blob layout (columns, per partition), consumption-ordered ----
# [ ltv_j0 | lstb_j0 | ltv_j1 | lstb_j1 ]                    header: 2048
# then per R-chunk ch (8 chunks of 512 cols, covering W k-tiles 4ch..4ch+3):
#   [ rv_j0 | rs_j0 | rv_j1 | rs_j1 ]                        2048
#   [ qc_{4ch} | qb_{4ch} | qc_{4ch+1} | qb_{4ch+1} ]        2048
#   [ qc_{4ch+2} | qb_{4ch+2} | qc_{4ch+3} | qb_{4ch+3} ]    2048
RCH = 8
RCW = D_IN // RCH         # 512 R columns per chunk
HDR = RG * 2 * OC         # 2048
SEG = 3 * 2048            # per-chunk segment
WBCOLS = HDR + RCH * SEG  # 51200


def _rv_off(j, ch):
    return HDR + ch * SEG + j * 2 * RCW


def _rs_off(j, ch):
    return _rv_off(j, ch) + RCW


def _qc_off(k):
    return HDR + (k // 4) * SEG + 2048 + (k % 4) * 2 * OC


def _qb_off(k):
    return _qc_off(k) + OC

_module_cache = {}
last_result = None


def _build_module(dt_name):
    import concourse.mybir as mybir
    import concourse.tile as tile
    from concourse import bacc

    use_f32r = dt_name == "float32r"
    dt_c = getattr(mybir.dt, dt_name)
    f32 = mybir.dt.float32

    def mm(ap):
        return ap

    nc = bacc.Bacc(None, target_bir_lowering=False, debug=False)
    xt_d = nc.dram_tensor("xt", (NS, KT, P, MS), dt_c, kind="ExternalInput")
    wb_d = nc.dram_tensor("wb", (P, WBCOLS), dt_c, kind="ExternalInput")
    bias_d = nc.dram_tensor("biasv", (P, OC), f32, kind="ExternalInput")
    y_d = nc.dram_tensor("y", (TOK, OC), f32, kind="ExternalOutput")

    with tile.TileContext(nc) as tc:
        with (
            tc.tile_pool(name="const", bufs=1) as const,
            tc.tile_pool(name="wpool", bufs=1) as wpool,
            tc.tile_pool(name="xpool", bufs=12) as xpool,
            tc.tile_pool(name="qpool", bufs=4) as qpool,
            tc.tile_pool(name="ypool", bufs=8) as ypool,
            tc.tile_pool(name="ppool", bufs=6, space="PSUM") as ppool,
            tc.tile_pool(name="wbpool", bufs=2, space="PSUM") as wbpool,
        ):
            # In f32r mode only the header+R pieces stay SBUF-resident
            # (budget); Q pieces stream through qpool inside build_w instead.
            rseg = 2048 if use_f32r else SEG
            WB = const.tile([P, HDR + RCH * rseg], dt_c)
            bias_t = const.tile([P, OC], f32)

            def ltv(j):
                return WB[:, j * 2 * OC:j * 2 * OC + OC]

            def lst(j):
                return WB[:, j * 2 * OC + OC:(j + 1) * 2 * OC]

            def rv(j, ch):
                o = HDR + ch * rseg + j * 2 * RCW
                return WB[:, o:o + RCW]

            def rs(j, ch):
                o = HDR + ch * rseg + j * 2 * RCW + RCW
                return WB[:, o:o + RCW]

            def qc(k):
                return WB[:, _qc_off(k):_qc_off(k) + OC]

            def qb(k):
                return WB[:, _qb_off(k):_qb_off(k) + OC]

            # blob streams in consumption order as 0.5 MB pieces
            nc.sync.dma_start(WB[:, 0:HDR], wb_d[:, 0:HDR])
            for ch in range(RCH):
                for po in range(0, rseg, 2048):
                    nc.sync.dma_start(
                        WB[:, HDR + ch * rseg + po:HDR + ch * rseg + po + 2048],
                        wb_d[:, HDR + ch * SEG + po:HDR + ch * SEG + po + 2048],
                    )
            nc.sync.dma_start(bias_t[:], bias_d[:])

            # ---- dequantize L^T and R (codes x pre-broadcast scales).
            # R dequantizes in place over its code slice in the blob.
            LdT = const.tile([P, RG, OC], dt_c)
            for j in range(RG):
                nc.vector.tensor_mul(LdT[:, j, :], ltv(j), lst(j))

            def dequant_r(ch):
                # deferred per-chunk so the in-order DVE stream never blocks
                # the first W evictions on late R-chunk DMAs
                for j in range(RG):
                    nc.vector.tensor_mul(rv(j, ch), rv(j, ch), rs(j, ch))

            def rd(j, k):
                # dequantized R columns for W k-tile k (128 cols)
                base = HDR + (k // 4) * rseg + j * 2 * RCW + (k % 4) * P
                return WB[:, base:base + P]

            dequant_r(0)

            # ---- W_c = R^T @ L^T + Q^T, built one k-tile at a time.
            # The build is interleaved into slab 0's k-loop two tiles ahead
            # (build W[k+2] while slab 0 multiplies with W[k]) so the
            # DVE-bound build chain (~1.25us/k) hides under PE matmul work.
            Wt = wpool.tile([P, KT, OC], dt_c)

            def build_w(k):
                ps = wbpool.tile([P, OC], f32, tag="wb", name=f"wb{k}")
                for j in range(RG):
                    nc.tensor.matmul(
                        ps[:],
                        mm(rd(j, k)),
                        mm(LdT[:, j, :]),
                        start=(j == 0),
                        stop=(j == RG - 1),
                    )
                if use_f32r:
                    qt = qpool.tile([P, 2 * OC], dt_c, tag="qt")
                    nc.sync.dma_start(qt[:], wb_d[:, _qc_off(k):_qc_off(k) + 2 * OC])
                    qc_ap, qb_ap = qt[:, :OC], qt[:, OC:]
                else:
                    qc_ap, qb_ap = qc(k), qb(k)
                qdq = qpool.tile([P, OC], dt_c, tag="qd")
                nc.vector.tensor_mul(qdq[:], qc_ap, qb_ap)
                nc.vector.tensor_add(Wt[:, k, :], ps[:], qdq[:])

            def evict(psums, s):
                for sub in range(SUB):
                    yt = ypool.tile([P, OC], f32, tag="y", name=f"y{s}_{sub}")
                    nc.vector.tensor_add(yt[:], psums[sub][:], bias_t[:])
                    nc.scalar.dma_start(
                        y_d[s * MS + sub * P:s * MS + (sub + 1) * P, :], yt[:]
                    )

            for _k in range(3):
                build_w(_k)
            psums0 = [ppool.tile([P, OC], f32, tag="ps", name=f"ps0_{i}")
                      for i in range(SUB)]
            for k in range(KT):
                xt = xpool.tile([P, MS], dt_c, tag="x", name="xt0")
                nc.scalar.dma_start(xt[:], xt_d[0, k])
                for sub in range(SUB):
                    nc.tensor.matmul(
                        psums0[sub][:], mm(xt[:, sub * P:(sub + 1) * P]),
                        mm(Wt[:, k, :]), start=(k == 0), stop=(k == KT - 1),
                    )
                if k + 3 < KT:
                    if (k + 3) % (KT // RCH) == 0:
                        dequant_r((k + 3) // (KT // RCH))
                    build_w(k + 3)
                # KT//RCH == 4: chunk ch feeds W k-tiles 4ch..4ch+3
            evict(psums0, 0)

            for s in range(1, NS):
                psums = [
                    ppool.tile([P, OC], f32, tag="ps", name=f"ps{s}_{i}")
                    for i in range(SUB)
                ]
                for k in range(KT):
                    xt = xpool.tile([P, MS], dt_c, tag="x")
                    dma_eng = nc.sync if k % 2 == 0 else nc.scalar
                    dma_eng.dma_start(xt[:], xt_d[s, k])
                    for sub in range(SUB):
                        nc.tensor.matmul(
                            psums[sub][:],
                            mm(xt[:, sub * P:(sub + 1) * P]),
                            mm(Wt[:, k, :]),
                            start=(k == 0),
                            stop=(k == KT - 1),
                        )
                evict(psums, s)

    nc.compile()
    return nc


def kernel(x, q_values, q_scales, l_values, l_scales, r_values, r_scales, bias,
           _trace=False):
    from concourse.bass_utils import run_bass_kernel_spmd

    dt_name = os.environ.get("CALDERA_DTYPE", "bfloat16")
    np_in = ml_dtypes.bfloat16 if dt_name == "bfloat16" else np.float32

    if dt_name not in _module_cache:
        _module_cache[dt_name] = _build_module(dt_name)
    nc = _module_cache[dt_name]

    # host-side marshaling (layout + dtype only; all math runs on-device)
    x = np.asarray(x, dtype=np.float32)
    q_values = np.asarray(q_values)
    q_scales = np.asarray(q_scales)
    l_values = np.asarray(l_values)
    l_scales = np.asarray(l_scales)
    r_values = np.asarray(r_values)
    r_scales = np.asarray(r_scales)
    bias = np.asarray(bias)
    # xt[s, k, p, m] = x[s*MS + m, k*P + p]
    xt = np.ascontiguousarray(
        x.reshape(NS, MS, KT, P).transpose(0, 2, 3, 1)
    ).astype(np_in)
    rs_full = np.repeat(np.asarray(r_scales, np.float32), D_IN // KG, axis=1)
    rv_f = np.asarray(r_values, np.float32)

    in_maps = []
    for c in range(NCORES):
        sl = slice(c * OC, (c + 1) * OC)
        qt_c = q_values[sl].T.astype(np.float32)           # [D_IN, OC]
        qst_c = q_scales[sl].T.astype(np.float32)          # [KT, OC]
        ltv_c = l_values[sl].T.astype(np.float32)          # [RANK, OC]
        lst_c = l_scales[sl].T.astype(np.float32)          # [RG, OC]

        pieces = []
        for j in range(RG):
            pieces.append(ltv_c[j * P:(j + 1) * P, :])
            pieces.append(np.broadcast_to(lst_c[j].reshape(1, OC), (P, OC)))
        for ch in range(RCH):
            cs = slice(ch * RCW, (ch + 1) * RCW)
            for j in range(RG):
                pieces.append(rv_f[j * P:(j + 1) * P, cs])
                pieces.append(rs_full[j * P:(j + 1) * P, cs])
            for k in range(4 * ch, 4 * ch + 4):
                pieces.append(qt_c[k * P:(k + 1) * P, :])
                pieces.append(np.broadcast_to(qst_c[k].reshape(1, OC), (P, OC)))
        wb = np.concatenate(pieces, axis=1).astype(np_in)
        assert wb.shape == (P, WBCOLS)

        in_maps.append({
            "xt": xt,
            "wb": wb,
            "biasv": np.ascontiguousarray(
                np.broadcast_to(bias[sl].reshape(1, OC), (P, OC))
            ).astype(np.float32),
        })

    res = run_bass_kernel_spmd(
        nc, in_maps, core_ids=list(range(NCORES)), trace=_trace
    )
    global last_result
    last_result = res
    return np.concatenate([r["y"] for r in res.results], axis=1)


# revision 56
# speedup vs baseline: 1.0036x; 1.0036x over previous
"""CalderaLinear fused kernel for 8 Trainium2 NeuronCores.

Math (reference): y = x @ Q^T + (x @ R^T) @ L^T + bias, with Q/L/R groupwise
int-dequantized (codes 0..15, group size 128).

Strategy:
  * Column-parallel over d_out: core c owns out-features [c*512, (c+1)*512).
  * On each core, first build W_c = Q_c^T + R^T @ L_c^T  ([d_in, 512]) on-chip:
    R and L^T are dequantized with DVE multiplies (codes and pre-broadcast
    scales arrive as exact bf16), R^T L^T accumulates on the PE into PSUM, and
    dequantized Q^T is added during PSUM eviction into the resident W tile.
  * Then y_c = x @ W_c + bias_c: x streams through as 128x512 pre-tiled
    blocks (host-side retiling gives one contiguous DMA per tile), W_c stays
    SBUF-resident, PSUM accumulates over the 32 k-tiles, bias is fused into
    the PSUM eviction.
  * All W-build operands are packed host-side into one contiguous
    [128, 51200] blob so the build phase streams in as a handful of large
    DMAs (the per-tensor version paid ~2.5us of cold-queue latency per small
    DMA at kernel start).
  * Host side only reshapes/transposes/casts and concatenates the 8 output
    shards: all dequant + matmul math runs on the NeuronCores.

Compute dtype is bf16 (codes 0..15 are exact; rel-err ~3e-3 vs fp32
reference, dominated by bf16 rounding of x and W). Set CALDERA_DTYPE=float32r
for the reduced-precision-fp32 PE mode (~2e-4 rel-err, ~15% slower).
"""

import os
import numpy as np
import ml_dtypes

P = 128
D_IN = 4096
D_OUT = 4096
TOK = 8192
RANK = 256
NCORES = 8
OC = D_OUT // NCORES      # 512 out features per core
KT = D_IN // P            # 32 contraction tiles
MS = 512                  # token slab
NS = TOK // MS            # 16 slabs
SUB = MS // P             # 4 psum sub-tiles per slab
KG = D_IN // 128          # 32 scale groups along d_in
RG = RANK // 128          # 2 scale groups along rank

# ---- W-build blob layout (columns, per partition), consumption-ordered ----
# [ ltv_j0 | lstb_j0 | ltv_j1 | lstb_j1 ]                    header: 2048
# then per R-chunk ch (8 chunks of 512 cols, covering W k-tiles 4ch..4ch+3):
#   [ rv_j0 | rs_j0 | rv_j1 | rs_j1 ]                        2048
#   [ qc_{4ch} | qb_{4ch} | qc_{4ch+1} | qb_{4ch+1} ]        2048
#   [ qc_{4ch+2} | qb_{4ch+2} | qc_{4ch+3} | qb_{4ch+3} ]    2048
RCH = 8
RCW = D_IN // RCH         # 512 R columns per chunk
HDR = RG * 2 * OC         # 2048
SEG = 3 * 2048            # per-chunk segment
WBCOLS = HDR + RCH * SEG  # 51200


def _rv_off(j, ch):
    return HDR + ch * SEG + j * 2 * RCW


def _rs_off(j, ch):
    return _rv_off(j, ch) + RCW


def _qc_off(k):
    return HDR + (k // 4) * SEG + 2048 + (k % 4) * 2 * OC


def _qb_off(k):
    return _qc_off(k) + OC

_module_cache = {}
last_result = None


def _build_module(dt_name):
    import concourse.mybir as mybir
    import concourse.tile as tile
    from concourse import bacc

    use_f32r = dt_name == "float32r"
    dt_c = getattr(mybir.dt, dt_name)
    f32 = mybir.dt.float32

    def mm(ap):
        return ap

    nc = bacc.Bacc(None, target_bir_lowering=False, debug=False)
    xt_d = nc.dram_tensor("xt", (NS, KT, P, MS), dt_c, kind="ExternalInput")
    wb_d = nc.dram_tensor("wb", (P, WBCOLS), dt_c, kind="ExternalInput")
    bias_d = nc.dram_tensor("biasv", (P, OC), f32, kind="ExternalInput")
    y_d = nc.dram_tensor("y", (TOK, OC), f32, kind="ExternalOutput")

    with tile.TileContext(nc) as tc:
        with (
            tc.tile_pool(name="const", bufs=1) as const,
            tc.tile_pool(name="wpool", bufs=1) as wpool,
            tc.tile_pool(name="xpool", bufs=16) as xpool,
            tc.tile_pool(name="qpool", bufs=4) as qpool,
            tc.tile_pool(name="ypool", bufs=8) as ypool,
            tc.tile_pool(name="ppool", bufs=6, space="PSUM") as ppool,
            tc.tile_pool(name="wbpool", bufs=2, space="PSUM") as wbpool,
        ):
            # In f32r mode only the header+R pieces stay SBUF-resident
            # (budget); Q pieces stream through qpool inside build_w instead.
            rseg = 2048 if use_f32r else SEG
            WB = const.tile([P, HDR + RCH * rseg], dt_c)
            bias_t = const.tile([P, OC], f32)

            def ltv(j):
                return WB[:, j * 2 * OC:j * 2 * OC + OC]

            def lst(j):
                return WB[:, j * 2 * OC + OC:(j + 1) * 2 * OC]

            def rv(j, ch):
                o = HDR + ch * rseg + j * 2 * RCW
                return WB[:, o:o + RCW]

            def rs(j, ch):
                o = HDR + ch * rseg + j * 2 * RCW + RCW
                return WB[:, o:o + RCW]

            def qc(k):
                return WB[:, _qc_off(k):_qc_off(k) + OC]

            def qb(k):
                return WB[:, _qb_off(k):_qb_off(k) + OC]

            # blob streams in consumption order as 0.5 MB pieces
            nc.sync.dma_start(WB[:, 0:HDR], wb_d[:, 0:HDR])
            for ch in range(RCH):
                for po in range(0, rseg, 2048):
                    nc.sync.dma_start(
                        WB[:, HDR + ch * rseg + po:HDR + ch * rseg + po + 2048],
                        wb_d[:, HDR + ch * SEG + po:HDR + ch * SEG + po + 2048],
                    )
            nc.sync.dma_start(bias_t[:], bias_d[:])

            # ---- dequantize L^T and R (codes x pre-broadcast scales).
            # R dequantizes in place over its code slice in the blob.
            LdT = const.tile([P, RG, OC], dt_c)
            for j in range(RG):
                nc.vector.tensor_mul(LdT[:, j, :], ltv(j), lst(j))

            def dequant_r(ch):
                # deferred per-chunk so the in-order DVE stream never blocks
                # the first W evictions on late R-chunk DMAs
                for j in range(RG):
                    nc.vector.tensor_mul(rv(j, ch), rv(j, ch), rs(j, ch))

            def rd(j, k):
                # dequantized R columns for W k-tile k (128 cols)
                base = HDR + (k // 4) * rseg + j * 2 * RCW + (k % 4) * P
                return WB[:, base:base + P]

            dequant_r(0)

            # ---- W_c = R^T @ L^T + Q^T, built one k-tile at a time.
            # The build is interleaved into slab 0's k-loop two tiles ahead
            # (build W[k+2] while slab 0 multiplies with W[k]) so the
            # DVE-bound build chain (~1.25us/k) hides under PE matmul work.
            Wt = wpool.tile([P, KT, OC], dt_c)

            def build_w(k):
                ps = wbpool.tile([P, OC], f32, tag="wb", name=f"wb{k}")
                for j in range(RG):
                    nc.tensor.matmul(
                        ps[:],
                        mm(rd(j, k)),
                        mm(LdT[:, j, :]),
                        start=(j == 0),
                        stop=(j == RG - 1),
                    )
                if use_f32r:
                    qt = qpool.tile([P, 2 * OC], dt_c, tag="qt")
                    nc.sync.dma_start(qt[:], wb_d[:, _qc_off(k):_qc_off(k) + 2 * OC])
                    qc_ap, qb_ap = qt[:, :OC], qt[:, OC:]
                else:
                    qc_ap, qb_ap = qc(k), qb(k)
                qdq = qpool.tile([P, OC], dt_c, tag="qd")
                nc.vector.tensor_mul(qdq[:], qc_ap, qb_ap)
                nc.vector.tensor_add(Wt[:, k, :], ps[:], qdq[:])

            def evict(psums, s):
                for sub in range(SUB):
                    yt = ypool.tile([P, OC], f32, tag="y", name=f"y{s}_{sub}")
                    nc.vector.tensor_add(yt[:], psums[sub][:], bias_t[:])
                    nc.scalar.dma_start(
                        y_d[s * MS + sub * P:s * MS + (sub + 1) * P, :], yt[:]
                    )

            for _k in range(3):
                build_w(_k)
            psums0 = [ppool.tile([P, OC], f32, tag="ps", name=f"ps0_{i}")
                      for i in range(SUB)]
            for k in range(KT):
                xt = xpool.tile([P, MS], dt_c, tag="x", name="xt0")
                nc.scalar.dma_start(xt[:], xt_d[0, k])
                for sub in range(SUB):
                    nc.tensor.matmul(
                        psums0[sub][:], mm(xt[:, sub * P:(sub + 1) * P]),
                        mm(Wt[:, k, :]), start=(k == 0), stop=(k == KT - 1),
                    )
                if k + 3 < KT:
                    if (k + 3) % (KT // RCH) == 0:
                        dequant_r((k + 3) // (KT // RCH))
                    build_w(k + 3)
                # KT//RCH == 4: chunk ch feeds W k-tiles 4ch..4ch+3
            evict(psums0, 0)

            for s in range(1, NS):
                psums = [
                    ppool.tile([P, OC], f32, tag="ps", name=f"ps{s}_{i}")
                    for i in range(SUB)
                ]
                for k in range(KT):
                    xt = xpool.tile([P, MS], dt_c, tag="x")
                    dma_eng = nc.sync if k % 2 == 0 else nc.scalar
                    dma_eng.dma_start(xt[:], xt_d[s, k])
                    for sub in range(SUB):
                        nc.tensor.matmul(
                            psums[sub][:],
                            mm(xt[:, sub * P:(sub + 1) * P]),
                            mm(Wt[:, k, :]),
                            start=(k == 0),
                            stop=(k == KT - 1),
                        )
                evict(psums, s)

    nc.compile()
    return nc


def kernel(x, q_values, q_scales, l_values, l_scales, r_values, r_scales, bias,
           _trace=False):
    from concourse.bass_utils import run_bass_kernel_spmd

    dt_name = os.environ.get("CALDERA_DTYPE", "bfloat16")
    np_in = ml_dtypes.bfloat16 if dt_name == "bfloat16" else np.float32

    if dt_name not in _module_cache:
        _module_cache[dt_name] = _build_module(dt_name)
    nc = _module_cache[dt_name]

    # host-side marshaling (layout + dtype only; all math runs on-device)
    x = np.asarray(x, dtype=np.float32)
    q_values = np.asarray(q_values)
    q_scales = np.asarray(q_scales)
    l_values = np.asarray(l_values)
    l_scales = np.asarray(l_scales)
    r_values = np.asarray(r_values)
    r_scales = np.asarray(r_scales)
    bias = np.asarray(bias)
    # xt[s, k, p, m] = x[s*MS + m, k*P + p]
    xt = np.ascontiguousarray(
        x.reshape(NS, MS, KT, P).transpose(0, 2, 3, 1)
    ).astype(np_in)
    rs_full = np.repeat(np.asarray(r_scales, np.float32), D_IN // KG, axis=1)
    rv_f = np.asarray(r_values, np.float32)

    in_maps = []
    for c in range(NCORES):
        sl = slice(c * OC, (c + 1) * OC)
        qt_c = q_values[sl].T.astype(np.float32)           # [D_IN, OC]
        qst_c = q_scales[sl].T.astype(np.float32)          # [KT, OC]
        ltv_c = l_values[sl].T.astype(np.float32)          # [RANK, OC]
        lst_c = l_scales[sl].T.astype(np.float32)          # [RG, OC]

        pieces = []
        for j in range(RG):
            pieces.append(ltv_c[j * P:(j + 1) * P, :])
            pieces.append(np.broadcast_to(lst_c[j].reshape(1, OC), (P, OC)))
        for ch in range(RCH):
            cs = slice(ch * RCW, (ch + 1) * RCW)
            for j in range(RG):
                pieces.append(rv_f[j * P:(j + 1) * P, cs])
                pieces.append(rs_full[j * P:(j + 1) * P, cs])
            for k in range(4 * ch, 4 * ch + 4):
                pieces.append(qt_c[k * P:(k + 1) * P, :])
                pieces.append(np.broadcast_to(qst_c[k].reshape(1, OC), (P, OC)))
        wb = np.concatenate(pieces, axis=1).astype(np_in)
        assert wb.shape == (P, WBCOLS)

        in_maps.append({
            "xt": xt,
            "wb": wb,
            "biasv": np.ascontiguousarray(
                np.broadcast_to(bias[sl].reshape(1, OC), (P, OC))
            ).astype(np.float32),
        })

    res = run_bass_kernel_spmd(
        nc, in_maps, core_ids=list(range(NCORES)), trace=_trace
    )
    global last_result
    last_result = res
    return np.concatenate([r["y"] for r in res.results], axis=1)
